# revision 21
# baseline (speedup 1.0000x reference)
"""MASKGCN Trainium2 kernel: 3-layer masked GCN over B=512 graphs of N=200 nodes.

Strategy
--------
Data-parallel over the batch: 64 graphs per NeuronCore, 8 cores, no collectives.

Math fold (exact up to fp reassociation):
    mask = (E + E^T)/2 + I                 (host)
    A    = sigmoid(adj) * mask             (device; adj is 0/1 so
                                            sigmoid(adj) = c*(adj + s), s = 0.5/c,
                                            c = sigmoid(1) - 0.5)
    S0 = F @ W0 ; H1 = A @ S0 ; S1 = H1 @ W1 ; H2 = A @ S1
    out_g = (1/200) * colsum(A)^T @ (H2 @ (W2 @ pw)) + pb
The whole last GCN layer + mean readout + linear head collapse into a
[200]-vector / [256,2]-matrix contraction (colsum(A) is a free-axis reduction
of A^T, fused into the same op that builds A^T).

Precision: all device tensors are fp16 (PE runs fp16 at 1 col/cycle vs
fp32's 4; PSUM accumulation stays fp32). Simulated end-to-end rel-norm
error vs the fp32 reference: ~9e-4.

Layouts: host passes adj^T and F^T per graph. A^T = sigmoid(adj^T) * mask
(mask symmetric). The chain alternates normal/transposed layouts so that NO
on-chip transposes are needed:
    S0  (normal  [node, h])  = matmul(lhsT=F^T slices,  rhs=W0)
    H1t (transp. [h, node])  = matmul(lhsT=S0 slices,   rhs=A^T)
    S1  (normal)             = matmul(lhsT=H1t slices,  rhs=W1)
    H2t (transp.)            = matmul(lhsT=S1 slices,   rhs=A^T)
    S2p (normal [node, 2])   = matmul(lhsT=H2t slices,  rhs=Wp)   Wp=(W2@pw)/200
    og  ([1, 2])             = matmul(lhsT=w,           rhs=S2p)  w=colsum(A)

Engine split: GpSimd builds A^T (SBUF->SBUF, fused colsum accum), Vector
evacuates S0/S1 (PSUM->SBUF), Scalar evacuates H1t/H2t.
"""

import os
import sys
import numpy as np

# concourse is normally pre-imported by the axon sitecustomize; these are
# fallbacks for environments where it is not on the default path.
if "concourse" not in sys.modules:
    try:
        import concourse  # noqa: F401
    except ImportError:
        for _p in ("/opt/trn_rl_repo", "/root/.axon_site/_ro/trn_rl_repo"):
            if os.path.isdir(_p) and _p not in sys.path:
                sys.path.append(_p)

B, N, IN_C, HID, OUT_C, N_VARS = 512, 200, 200, 256, 256, 2
N_CORES = 8
BPC = B // N_CORES  # graphs per core
P0 = 128
P1 = N - P0  # 72

# sigmoid(adj) = C_SIG * (adj + S_SIG) for adj in {0, 1}
C_SIG = float(1.0 / (1.0 + np.exp(-1.0)) - 0.5)  # 0.23105857863000487
S_SIG = float(0.5 / C_SIG)                       # 2.1639534137386535

_BUILD_CACHE = {}


def _build_nc(bpc, reps=1):
    """Build the per-core Bass program (SPMD: identical on all cores).

    reps>1 wraps the whole batch loop in a hardware For_i so the body runs
    `reps` times — benchmarking only (differencing cancels dispatch floor)."""
    import concourse.bacc as bacc
    import concourse.mybir as mybir
    import concourse.tile as tile
    from contextlib import ExitStack

    f32 = mybir.dt.float32
    f16 = mybir.dt.float16
    ADD = mybir.AluOpType.add
    MULT = mybir.AluOpType.mult

    nc = bacc.Bacc(None, target_bir_lowering=False)
    # "adjt" carries host-prebuilt A^T = sigmoid(adj^T)*mask (fp16); "wvt"
    # carries host-prebuilt colsum(A) packed as [128, 2] per graph.
    adjt = nc.declare_dram_parameter("adjt", [bpc, N, N], f16, isOutput=False)
    ft = nc.declare_dram_parameter("ft", [bpc, N, N], f16, isOutput=False)
    wvt = nc.declare_dram_parameter("wvt", [bpc, P0, 2], f16, isOutput=False)
    w0 = nc.declare_dram_parameter("w0", [IN_C, HID], f16, isOutput=False)
    w1 = nc.declare_dram_parameter("w1", [HID, HID], f16, isOutput=False)
    wp = nc.declare_dram_parameter("wp", [OUT_C, N_VARS], f16, isOutput=False)
    out = nc.declare_dram_parameter("out", [1, bpc * N_VARS], f32, isOutput=True)

    with tile.TileContext(nc) as tc, ExitStack() as ctx:
        consts = ctx.enter_context(tc.tile_pool(name="consts", bufs=1))
        inp = ctx.enter_context(tc.tile_pool(name="inp", bufs=8))
        sp = ctx.enter_context(tc.tile_pool(name="sp", bufs=3))
        htp = ctx.enter_context(tc.tile_pool(name="htp", bufs=3))
        smallp = ctx.enter_context(tc.tile_pool(name="smallp", bufs=4))
        pstage = ctx.enter_context(
            tc.tile_pool(name="pstage", bufs=2, space="PSUM")
        )

        # ---- constants (loaded once) ----
        w0a = consts.tile([P0, HID], f16, tag="w0a")
        w0b = consts.tile([P1, HID], f16, tag="w0b")
        w1a = consts.tile([P0, HID], f16, tag="w1a")
        w1b = consts.tile([P0, HID], f16, tag="w1b")
        wpa = consts.tile([P0, N_VARS], f16, tag="wpa")
        wpb = consts.tile([P0, N_VARS], f16, tag="wpb")
        out_acc = consts.tile([1, bpc * N_VARS], f32, tag="out_acc")
        # w0 first: the first graph's S0 needs it; w1/wp aren't read until
        # ~2us later, so they queue behind the first graph DMAs.
        nc.sync.dma_start(w0a[:], w0[0:P0, :])
        nc.sync.dma_start(w0b[:], w0[P0:N, :])
        w0_t = (w0a, w0b)
        w1_t = (w1a, w1b)
        wp_t = (wpa, wpb)

        mslc = ((0, P0), (P0, P1))  # node-dim (offset, count) tiles

        def mm_split(out_ap, lhsT_ap, rhs_ap, start, stop):
            """Emit LDWEIGHTS + non-self-loading MATMUL so consecutive
            weight loads pipeline through the PE reorder window instead of
            serializing at the head of every matmul."""
            nc.tensor.ldweights(lhsT_ap)
            inst = nc.tensor.matmul(
                out_ap, lhsT_ap, rhs_ap, start=start, stop=stop
            )
            inst.ldweights = False
            return inst

        def emit_batch():
            state = {}
            late_consts = [False]

            def st_late_consts():
                # Issued after the first pair's input DMAs so they don't
                # delay the first matmul.
                nc.sync.dma_start(w1a[:], w1[0:P0, :])
                nc.sync.dma_start(w1b[:], w1[P0:HID, :])
                nc.sync.dma_start(wpa[:], wp[0:P0, :])
                nc.sync.dma_start(wpb[:], wp[P0:OUT_C, :])

            def st_dma(g):
                t = {}
                t["f0"] = inp.tile([P0, N], f16, tag="f0", name="f0")
                t["f1"] = inp.tile([P1, N], f16, tag="f1", name="f1")
                at0 = inp.tile([P0, N], f16, tag="at0", name="at0")
                at1 = inp.tile([P1, N], f16, tag="at1", name="at1")
                wv = inp.tile([P0, 2], f16, tag="wv", name="wv")
                nc.sync.dma_start(t["f0"][:], ft[g, 0:P0, :])
                nc.sync.dma_start(t["f1"][:], ft[g, P0:N, :])
                nc.sync.dma_start(at0[:], adjt[g, 0:P0, :])
                nc.sync.dma_start(at1[:], adjt[g, P0:N, :])
                nc.sync.dma_start(wv[:], wvt[g, :, :])
                t["at"] = (at0, at1)
                t["wv"] = wv
                state[g] = t

            def st_s0(g):
                # S0 = F @ W0 -> psum [node, 2*HID]; single fp16 evacuation
                t = state[g]
                ps0 = pstage.tile([P0, 2 * HID], f32, tag="ps0")
                f_t = (t["f0"], t["f1"])
                for j, (mo, mc) in enumerate(mslc):
                    for k in range(2):
                        nc.tensor.matmul(
                            ps0[0:mc, j * HID:(j + 1) * HID],
                            f_t[k][:, mo:mo + mc],
                            w0_t[k][:],
                            start=(k == 0), stop=(k == 1),
                        )
                # s01 holds [nodes0:128 x hid | nodes128:200 x hid]; the
                # copy drags along garbage rows 72:128 of the second half.
                s01 = sp.tile([P0, 2 * HID], f16, tag="s01")
                nc.vector.tensor_copy(s01[:], ps0[:])
                t["s01"] = s01

            def st_h1(g):
                # H1^T = matmul(lhsT=S0 slices, rhs=A^T) -> psum [h, 2*N]
                t = state[g]
                ph1 = pstage.tile([P0, 2 * N], f32, tag="ph1")
                s01 = t["s01"]
                for j in range(2):  # h slice
                    for k, kc in ((0, P0), (1, P1)):  # node contraction tile
                        nc.tensor.matmul(
                            ph1[:, j * N:(j + 1) * N],
                            s01[0:kc, k * HID + j * P0:k * HID + (j + 1) * P0],
                            t["at"][k][:],
                            start=(k == 0), stop=(k == 1),
                        )
                h1 = htp.tile([P0, 2 * N], f16, tag="h1")
                nc.scalar.copy(h1[:], ph1[:])
                t["h1"] = h1

            def st_s1(g):
                # S1 = H1 @ W1 -> psum [node, 2*HID]
                t = state[g]
                ps1 = pstage.tile([P0, 2 * HID], f32, tag="ps1")
                h1 = t["h1"]
                for j, (mo, mc) in enumerate(mslc):
                    for k in range(2):  # hid contraction tile
                        nc.tensor.matmul(
                            ps1[0:mc, j * HID:(j + 1) * HID],
                            h1[:, k * N + mo:k * N + mo + mc],
                            w1_t[k][:],
                            start=(k == 0), stop=(k == 1),
                        )
                s11 = sp.tile([P0, 2 * HID], f16, tag="s11")
                nc.vector.tensor_copy(s11[:], ps1[:])
                t["s11"] = s11

            def st_h2(g):
                # H2^T -> psum phx[:, 0:400]; phx also hosts S2p (400:404)
                # and og (404:406) so the whole tail shares one PSUM bank.
                t = state[g]
                phx = pstage.tile([P0, 2 * N + 2 * N_VARS + N_VARS], f32,
                                  tag="phx")
                s11 = t["s11"]
                for j in range(2):
                    for k, kc in ((0, P0), (1, P1)):
                        nc.tensor.matmul(
                            phx[:, j * N:(j + 1) * N],
                            s11[0:kc, k * HID + j * P0:k * HID + (j + 1) * P0],
                            t["at"][k][:],
                            start=(k == 0), stop=(k == 1),
                        )
                h2 = htp.tile([P0, 2 * N], f16, tag="h2")
                nc.scalar.copy(h2[:], phx[:, 0:2 * N])
                t["h2"] = h2
                t["phx"] = phx

            def st_tail(g):
                # S2p = H2 @ Wp; og = colsum(A)^T @ S2p
                t = state[g]
                phx = t["phx"]
                h2 = t["h2"]
                c0 = 2 * N
                for j, (mo, mc) in enumerate(mslc):
                    for k in range(2):  # hid contraction tile
                        nc.tensor.matmul(
                            phx[0:mc, c0 + j * N_VARS:c0 + (j + 1) * N_VARS],
                            h2[:, k * N + mo:k * N + mo + mc],
                            wp_t[k][:],
                            start=(k == 0), stop=(k == 1),
                        )
                s2p = smallp.tile([P0, 2 * N_VARS], f16, tag="s2p")
                nc.scalar.copy(s2p[:], phx[:, c0:c0 + 2 * N_VARS])
                t["s2p"] = s2p

            def st_og(g):
                # Deferred one pair: by now the s2p copy has long
                # completed, so these matmuls never stall the PE queue.
                t = state[g]
                phx = t["phx"]
                s2p = t["s2p"]
                c1 = 2 * N + 2 * N_VARS
                wv = t["wv"]
                nc.tensor.matmul(
                    phx[0:1, c1:c1 + N_VARS], wv[:, 0:1], s2p[:, 0:N_VARS],
                    start=True, stop=False,
                )
                nc.tensor.matmul(
                    phx[0:1, c1:c1 + N_VARS], wv[0:P1, 1:2],
                    s2p[0:P1, N_VARS:2 * N_VARS],
                    start=False, stop=True,
                )
                nc.scalar.copy(
                    out_acc[:, g * N_VARS:(g + 1) * N_VARS],
                    phx[0:1, c1:c1 + N_VARS],
                )
                del state[g]

            # Two-graph software pipeline: stage X of graph g overlaps
            # stage X of graph g^1, so PSUM evacuations hide under the
            # partner graph's matmul stream. og matmuls run one pair late
            # so their wait-on-copy never blocks the PE queue.
            stages = (st_s0, st_h1, st_s1, st_h2, st_tail)
            for g0 in range(0, bpc, 2):
                g1 = g0 + 1
                st_dma(g0)
                st_dma(g1)
                if not late_consts[0]:
                    st_late_consts()
                    late_consts[0] = True
                first = True
                for st in stages:
                    st(g0)
                    if first and g0 >= 2:
                        st_og(g0 - 2)
                        st_og(g0 - 1)
                        first = False
                    st(g1)
            st_og(bpc - 2)
            st_og(bpc - 1)

        if reps > 1:
            with tc.For_i(0, reps, 1):
                emit_batch()
        else:
            emit_batch()

        nc.sync.dma_start(out[:], out_acc[:])

    nc.compile()
    return nc


def _host_prep(adj, features, raw_edge_weight, W0, W1, W2, pw, pb):
    """Host-side prep: build A^T = sigmoid(adj^T)*mask and colsum(A) here
    (same bytes as shipping adj^T, but saves the on-device mask multiply),
    plus fp16 weight shards."""
    mask = ((raw_edge_weight + raw_edge_weight.T) * 0.5
            + np.eye(N, dtype=np.float32)).astype(np.float32)
    wp = (W2.astype(np.float64) @ pw.astype(np.float64) / float(N)).astype(np.float16)
    w0h = W0.astype(np.float16)
    w1h = W1.astype(np.float16)
    # A^T[g] = (c*adj^T + 0.5) * mask  (mask is symmetric)
    adjt = np.ascontiguousarray(adj.transpose(0, 2, 1))
    at_all = ((np.float32(C_SIG) * adjt + np.float32(0.5)) * mask[None]
              ).astype(np.float16)
    # wv[g, m] = sum_n A^T[g, m, n] = colsum(A)[m]; packed [128, 2] with
    # column 0 = nodes 0:128, column 1 = nodes 128:200 (+ zero pad).
    wv = at_all.astype(np.float32).sum(axis=2)
    wvt_all = np.zeros((B, P0, 2), dtype=np.float16)
    wvt_all[:, :, 0] = wv[:, 0:P0]
    wvt_all[:, 0:P1, 1] = wv[:, P0:N]
    ft_all = np.ascontiguousarray(
        features.transpose(0, 2, 1).astype(np.float16)
    )
    in_maps = []
    for c in range(N_CORES):
        sl = slice(c * BPC, (c + 1) * BPC)
        in_maps.append({
            "adjt": at_all[sl],
            "ft": ft_all[sl],
            "wvt": wvt_all[sl],
            "w0": w0h,
            "w1": w1h,
            "wp": wp,
        })
    return in_maps


def _ensure_ntff_hook():
    """Wire the axon NTFF profile hook into antenv.axon_hooks if missing.

    The agent image's antenv package lacks axon_hooks, so bass_utils's
    trace path dies on import. trn_agent_boot has the ctypes hook
    implementation; expose it under the module name bass_utils expects.
    """
    import types

    try:
        from antenv.axon_hooks import get_axon_ntff_profile_hook  # noqa: F401
        return
    except ImportError:
        pass
    try:
        from trn_agent_boot.trn_boot import _ntff_profile_via_ctypes
        hook = _ntff_profile_via_ctypes("/opt/axon/libaxon_pjrt.so")
    except Exception:
        hook = None
    mod = types.ModuleType("antenv.axon_hooks")
    state = {"hook": hook}
    mod.get_axon_ntff_profile_hook = lambda: state["hook"]
    mod.set_axon_ntff_profile_hook = lambda h: state.__setitem__("hook", h)
    sys.modules["antenv.axon_hooks"] = mod
    import antenv

    antenv.axon_hooks = mod


def kernel(adj, features, raw_edge_weight, W0, W1, W2, pw, pb, _trace=False):
    from concourse.bass_utils import run_bass_kernel_spmd

    if _trace:
        _ensure_ntff_hook()

    adj = np.asarray(adj, dtype=np.float32)
    features = np.asarray(features, dtype=np.float32)
    raw_edge_weight = np.asarray(raw_edge_weight, dtype=np.float32)
    W0 = np.asarray(W0, dtype=np.float32)
    W1 = np.asarray(W1, dtype=np.float32)
    W2 = np.asarray(W2, dtype=np.float32)
    pw = np.asarray(pw, dtype=np.float32)
    pb = np.asarray(pb, dtype=np.float32)

    if "nc" not in _BUILD_CACHE:
        _BUILD_CACHE["nc"] = _build_nc(BPC)
    nc = _BUILD_CACHE["nc"]

    in_maps = _host_prep(adj, features, raw_edge_weight, W0, W1, W2, pw, pb)
    res = run_bass_kernel_spmd(
        nc, in_maps, core_ids=list(range(N_CORES)), trace=bool(_trace)
    )
    out = np.concatenate(
        [res.results[c]["out"].reshape(BPC, N_VARS) for c in range(N_CORES)], axis=0
    )
    out = out + pb[None, :].astype(np.float32)
    if _trace:
        return out, res
    return out


# revision 27
# speedup vs baseline: 1.3707x; 1.3707x over previous
"""MASKGCN Trainium2 kernel: 3-layer masked GCN over B=512 graphs of N=200 nodes.

Strategy
--------
Data-parallel over the batch: 64 graphs per NeuronCore, 8 cores, no collectives.

Math fold (exact up to fp reassociation):
    mask = (E + E^T)/2 + I                 (host)
    A    = sigmoid(adj) * mask             (device; adj is 0/1 so
                                            sigmoid(adj) = c*(adj + s), s = 0.5/c,
                                            c = sigmoid(1) - 0.5)
    S0 = F @ W0 ; H1 = A @ S0 ; S1 = H1 @ W1 ; H2 = A @ S1
    out_g = (1/200) * colsum(A)^T @ (H2 @ (W2 @ pw)) + pb
The whole last GCN layer + mean readout + linear head collapse into a
[200]-vector / [256,2]-matrix contraction (colsum(A) is a free-axis reduction
of A^T, fused into the same op that builds A^T).

Precision: all device tensors are fp16 (PE runs fp16 at 1 col/cycle vs
fp32's 4; PSUM accumulation stays fp32). Simulated end-to-end rel-norm
error vs the fp32 reference: ~9e-4.

Layouts: host passes adj^T and F^T per graph. A^T = sigmoid(adj^T) * mask
(mask symmetric). The chain alternates normal/transposed layouts so that NO
on-chip transposes are needed:
    S0  (normal  [node, h])  = matmul(lhsT=F^T slices,  rhs=W0)
    H1t (transp. [h, node])  = matmul(lhsT=S0 slices,   rhs=A^T)
    S1  (normal)             = matmul(lhsT=H1t slices,  rhs=W1)
    H2t (transp.)            = matmul(lhsT=S1 slices,   rhs=A^T)
    S2p (normal [node, 2])   = matmul(lhsT=H2t slices,  rhs=Wp)   Wp=(W2@pw)/200
    og  ([1, 2])             = matmul(lhsT=w,           rhs=S2p)  w=colsum(A)

Engine split: GpSimd builds A^T (SBUF->SBUF, fused colsum accum), Vector
evacuates S0/S1 (PSUM->SBUF), Scalar evacuates H1t/H2t.
"""

import os
import sys
import numpy as np

# concourse is normally pre-imported by the axon sitecustomize; these are
# fallbacks for environments where it is not on the default path.
if "concourse" not in sys.modules:
    try:
        import concourse  # noqa: F401
    except ImportError:
        for _p in ("/opt/trn_rl_repo", "/root/.axon_site/_ro/trn_rl_repo"):
            if os.path.isdir(_p) and _p not in sys.path:
                sys.path.append(_p)

B, N, IN_C, HID, OUT_C, N_VARS = 512, 200, 200, 256, 256, 2
N_CORES = 8
BPC = B // N_CORES  # graphs per core
P0 = 128
P1 = N - P0  # 72

# sigmoid(adj) = C_SIG * (adj + S_SIG) for adj in {0, 1}
C_SIG = float(1.0 / (1.0 + np.exp(-1.0)) - 0.5)  # 0.23105857863000487
S_SIG = float(0.5 / C_SIG)                       # 2.1639534137386535

_BUILD_CACHE = {}


def _build_nc(bpc, reps=1):
    """Build the per-core Bass program (SPMD: identical on all cores).

    reps>1 wraps the whole batch loop in a hardware For_i so the body runs
    `reps` times — benchmarking only (differencing cancels dispatch floor)."""
    import concourse.bacc as bacc
    import concourse.mybir as mybir
    import concourse.tile as tile
    from contextlib import ExitStack

    f32 = mybir.dt.float32
    f16 = mybir.dt.float16
    ADD = mybir.AluOpType.add
    MULT = mybir.AluOpType.mult

    nc = bacc.Bacc(None, target_bir_lowering=False)
    # "adjt" carries host-prebuilt A^T = sigmoid(adj^T)*mask (fp16); "wvt"
    # carries host-prebuilt colsum(A) packed as [128, 2] per graph.
    adjt = nc.declare_dram_parameter("adjt", [bpc, N, N], f16, isOutput=False)
    ft = nc.declare_dram_parameter("ft", [bpc, N, N], f16, isOutput=False)
    wvt = nc.declare_dram_parameter("wvt", [P0, bpc * 2], f16, isOutput=False)
    w0 = nc.declare_dram_parameter("w0", [IN_C, HID], f16, isOutput=False)
    w1 = nc.declare_dram_parameter("w1", [HID, HID], f16, isOutput=False)
    wp = nc.declare_dram_parameter("wp", [OUT_C, N_VARS], f16, isOutput=False)
    out = nc.declare_dram_parameter("out", [1, bpc * N_VARS], f32, isOutput=True)

    with tile.TileContext(nc) as tc, ExitStack() as ctx:
        consts = ctx.enter_context(tc.tile_pool(name="consts", bufs=1))
        inp = ctx.enter_context(tc.tile_pool(name="inp", bufs=8))
        sp = ctx.enter_context(tc.tile_pool(name="sp", bufs=3))
        htp = ctx.enter_context(tc.tile_pool(name="htp", bufs=3))
        smallp = ctx.enter_context(tc.tile_pool(name="smallp", bufs=4))
        pstage = ctx.enter_context(
            tc.tile_pool(name="pstage", bufs=2, space="PSUM")
        )

        # ---- constants (loaded once) ----
        w0a = consts.tile([P0, HID], f16, tag="w0a")
        w0b = consts.tile([P1, HID], f16, tag="w0b")
        w1a = consts.tile([P0, HID], f16, tag="w1a")
        w1b = consts.tile([P0, HID], f16, tag="w1b")
        wpa = consts.tile([P0, N_VARS], f16, tag="wpa")
        wpb = consts.tile([P0, N_VARS], f16, tag="wpb")
        out_acc = consts.tile([1, bpc * N_VARS], f32, tag="out_acc")
        # w0 first: the first graph's S0 needs it; w1/wp aren't read until
        # ~2us later, so they queue behind the first graph DMAs.
        wvall = consts.tile([P0, bpc * 2], f16, tag="wvall")
        nc.sync.dma_start(w0a[:], w0[0:P0, :])
        nc.sync.dma_start(w0b[:], w0[P0:N, :])
        nc.sync.dma_start(wvall[:], wvt[:, :])
        w0_t = (w0a, w0b)
        w1_t = (w1a, w1b)
        wp_t = (wpa, wpb)

        mslc = ((0, P0), (P0, P1))  # node-dim (offset, count) tiles

        def mm_split(out_ap, lhsT_ap, rhs_ap, start, stop):
            """Emit LDWEIGHTS + non-self-loading MATMUL so consecutive
            weight loads pipeline through the PE reorder window instead of
            serializing at the head of every matmul."""
            nc.tensor.ldweights(lhsT_ap)
            inst = nc.tensor.matmul(
                out_ap, lhsT_ap, rhs_ap, start=start, stop=stop
            )
            inst.ldweights = False
            return inst

        def emit_batch():
            state = {}
            late_consts = [False]

            def st_late_consts():
                # Issued after the first pair's input DMAs so they don't
                # delay the first matmul.
                nc.sync.dma_start(w1a[:], w1[0:P0, :])
                nc.sync.dma_start(w1b[:], w1[P0:HID, :])
                nc.sync.dma_start(wpa[:], wp[0:P0, :])
                nc.sync.dma_start(wpb[:], wp[P0:OUT_C, :])

            def st_dma(g):
                t = {}
                t["f0"] = inp.tile([P0, N], f16, tag="f0", name="f0")
                t["f1"] = inp.tile([P1, N], f16, tag="f1", name="f1")
                at0 = inp.tile([P0, N], f16, tag="at0", name="at0")
                at1 = inp.tile([P1, N], f16, tag="at1", name="at1")
                nc.sync.dma_start(t["f0"][:], ft[g, 0:P0, :])
                nc.sync.dma_start(t["f1"][:], ft[g, P0:N, :])
                nc.sync.dma_start(at0[:], adjt[g, 0:P0, :])
                nc.sync.dma_start(at1[:], adjt[g, P0:N, :])
                t["at"] = (at0, at1)
                state[g] = t

            def st_s0(g):
                # S0 = F @ W0 -> psum [node, 2*HID]; single fp16 evacuation
                t = state[g]
                ps0 = pstage.tile([P0, 2 * HID], f32, tag="ps0")
                f_t = (t["f0"], t["f1"])
                for j, (mo, mc) in enumerate(mslc):
                    for k in range(2):
                        nc.tensor.matmul(
                            ps0[0:mc, j * HID:(j + 1) * HID],
                            f_t[k][:, mo:mo + mc],
                            w0_t[k][:],
                            start=(k == 0), stop=(k == 1),
                        )
                # s01 holds [nodes0:128 x hid | nodes128:200 x hid]; the
                # copy drags along garbage rows 72:128 of the second half.
                s01 = sp.tile([P0, 2 * HID], f16, tag="s01")
                nc.vector.tensor_copy(s01[:], ps0[:])
                t["s01"] = s01

            def st_h1(g):
                # H1^T = matmul(lhsT=S0 slices, rhs=A^T) -> psum [h, 2*N]
                t = state[g]
                ph1 = pstage.tile([P0, 2 * N], f32, tag="ph1")
                s01 = t["s01"]
                for j in range(2):  # h slice
                    for k, kc in ((0, P0), (1, P1)):  # node contraction tile
                        nc.tensor.matmul(
                            ph1[:, j * N:(j + 1) * N],
                            s01[0:kc, k * HID + j * P0:k * HID + (j + 1) * P0],
                            t["at"][k][:],
                            start=(k == 0), stop=(k == 1),
                        )
                h1 = htp.tile([P0, 2 * N], f16, tag="h1")
                nc.scalar.copy(h1[:], ph1[:])
                t["h1"] = h1

            def st_s1(g):
                # S1 = H1 @ W1 -> psum [node, 2*HID]
                t = state[g]
                ps1 = pstage.tile([P0, 2 * HID], f32, tag="ps1")
                h1 = t["h1"]
                for j, (mo, mc) in enumerate(mslc):
                    for k in range(2):  # hid contraction tile
                        nc.tensor.matmul(
                            ps1[0:mc, j * HID:(j + 1) * HID],
                            h1[:, k * N + mo:k * N + mo + mc],
                            w1_t[k][:],
                            start=(k == 0), stop=(k == 1),
                        )
                s11 = sp.tile([P0, 2 * HID], f16, tag="s11")
                nc.vector.tensor_copy(s11[:], ps1[:])
                t["s11"] = s11

            def st_h2(g):
                # H2^T -> psum phx[:, 0:400]; phx also hosts S2p (400:404)
                # and og (404:406) so the whole tail shares one PSUM bank.
                t = state[g]
                phx = pstage.tile([P0, 2 * N + 2 * N_VARS + N_VARS], f32,
                                  tag="phx")
                s11 = t["s11"]
                for j in range(2):
                    for k, kc in ((0, P0), (1, P1)):
                        nc.tensor.matmul(
                            phx[:, j * N:(j + 1) * N],
                            s11[0:kc, k * HID + j * P0:k * HID + (j + 1) * P0],
                            t["at"][k][:],
                            start=(k == 0), stop=(k == 1),
                        )
                h2 = htp.tile([P0, 2 * N], f16, tag="h2")
                nc.scalar.copy(h2[:], phx[:, 0:2 * N])
                t["h2"] = h2
                t["phx"] = phx

            def st_tail(g):
                # S2p = H2 @ Wp; og = colsum(A)^T @ S2p
                t = state[g]
                phx = t["phx"]
                h2 = t["h2"]
                c0 = 2 * N
                for j, (mo, mc) in enumerate(mslc):
                    for k in range(2):  # hid contraction tile
                        nc.tensor.matmul(
                            phx[0:mc, c0 + j * N_VARS:c0 + (j + 1) * N_VARS],
                            h2[:, k * N + mo:k * N + mo + mc],
                            wp_t[k][:],
                            start=(k == 0), stop=(k == 1),
                        )
                s2p = smallp.tile([P0, 2 * N_VARS], f16, tag="s2p")
                nc.scalar.copy(s2p[:], phx[:, c0:c0 + 2 * N_VARS])
                t["s2p"] = s2p

            def st_og(g):
                # Deferred one pair: by now the s2p copy has long
                # completed, so these matmuls never stall the PE queue.
                t = state[g]
                phx = t["phx"]
                s2p = t["s2p"]
                c1 = 2 * N + 2 * N_VARS
                nc.tensor.matmul(
                    phx[0:1, c1:c1 + N_VARS], wvall[:, 2 * g:2 * g + 1],
                    s2p[:, 0:N_VARS],
                    start=True, stop=False,
                )
                nc.tensor.matmul(
                    phx[0:1, c1:c1 + N_VARS], wvall[0:P1, 2 * g + 1:2 * g + 2],
                    s2p[0:P1, N_VARS:2 * N_VARS],
                    start=False, stop=True,
                )
                nc.scalar.copy(
                    out_acc[:, g * N_VARS:(g + 1) * N_VARS],
                    phx[0:1, c1:c1 + N_VARS],
                )
                del state[g]

            # Two-graph software pipeline: stage X of graph g overlaps
            # stage X of graph g^1, so PSUM evacuations hide under the
            # partner graph's matmul stream. og matmuls run one pair late
            # so their wait-on-copy never blocks the PE queue.
            stages = (st_s0, st_h1, st_s1, st_h2, st_tail)
            for g0 in range(0, bpc, 2):
                g1 = g0 + 1
                st_dma(g0)
                st_dma(g1)
                if not late_consts[0]:
                    st_late_consts()
                    late_consts[0] = True
                first = True
                for st in stages:
                    st(g0)
                    if first and g0 >= 2:
                        st_og(g0 - 2)
                        st_og(g0 - 1)
                        first = False
                    st(g1)
            st_og(bpc - 2)
            st_og(bpc - 1)

        if reps > 1:
            with tc.For_i(0, reps, 1):
                emit_batch()
        else:
            emit_batch()

        nc.sync.dma_start(out[:], out_acc[:])

    nc.compile()
    return nc


def _host_prep(adj, features, raw_edge_weight, W0, W1, W2, pw, pb):
    """Host-side prep: build A^T = sigmoid(adj^T)*mask and colsum(A) here
    (same bytes as shipping adj^T, but saves the on-device mask multiply),
    plus fp16 weight shards."""
    mask = ((raw_edge_weight + raw_edge_weight.T) * 0.5
            + np.eye(N, dtype=np.float32)).astype(np.float32)
    wp = (W2.astype(np.float64) @ pw.astype(np.float64) / float(N)).astype(np.float16)
    w0h = W0.astype(np.float16)
    w1h = W1.astype(np.float16)
    # A^T[g] = (c*adj^T + 0.5) * mask  (mask is symmetric)
    adjt = np.ascontiguousarray(adj.transpose(0, 2, 1))
    at_all = ((np.float32(C_SIG) * adjt + np.float32(0.5)) * mask[None]
              ).astype(np.float16)
    # wv[g, m] = sum_n A^T[g, m, n] = colsum(A)[m]; packed [128, 2] per
    # graph (column 0 = nodes 0:128, column 1 = nodes 128:200 + zero pad),
    # then all graphs side by side: [128, bpc*2].
    wv = at_all.astype(np.float32).sum(axis=2)
    wvt_all = np.zeros((B, P0, 2), dtype=np.float16)
    wvt_all[:, :, 0] = wv[:, 0:P0]
    wvt_all[:, 0:P1, 1] = wv[:, P0:N]
    ft_all = np.ascontiguousarray(
        features.transpose(0, 2, 1).astype(np.float16)
    )
    in_maps = []
    for c in range(N_CORES):
        sl = slice(c * BPC, (c + 1) * BPC)
        in_maps.append({
            "adjt": at_all[sl],
            "ft": ft_all[sl],
            "wvt": np.ascontiguousarray(
                wvt_all[sl].transpose(1, 0, 2).reshape(P0, BPC * 2)
            ),
            "w0": w0h,
            "w1": w1h,
            "wp": wp,
        })
    return in_maps


def _ensure_ntff_hook():
    """Wire the axon NTFF profile hook into antenv.axon_hooks if missing.

    The agent image's antenv package lacks axon_hooks, so bass_utils's
    trace path dies on import. trn_agent_boot has the ctypes hook
    implementation; expose it under the module name bass_utils expects.
    """
    import types

    try:
        from antenv.axon_hooks import get_axon_ntff_profile_hook  # noqa: F401
        return
    except ImportError:
        pass
    try:
        from trn_agent_boot.trn_boot import _ntff_profile_via_ctypes
        hook = _ntff_profile_via_ctypes("/opt/axon/libaxon_pjrt.so")
    except Exception:
        hook = None
    mod = types.ModuleType("antenv.axon_hooks")
    state = {"hook": hook}
    mod.get_axon_ntff_profile_hook = lambda: state["hook"]
    mod.set_axon_ntff_profile_hook = lambda h: state.__setitem__("hook", h)
    sys.modules["antenv.axon_hooks"] = mod
    import antenv

    antenv.axon_hooks = mod


def kernel(adj, features, raw_edge_weight, W0, W1, W2, pw, pb, _trace=False):
    from concourse.bass_utils import run_bass_kernel_spmd

    if _trace:
        _ensure_ntff_hook()

    adj = np.asarray(adj, dtype=np.float32)
    features = np.asarray(features, dtype=np.float32)
    raw_edge_weight = np.asarray(raw_edge_weight, dtype=np.float32)
    W0 = np.asarray(W0, dtype=np.float32)
    W1 = np.asarray(W1, dtype=np.float32)
    W2 = np.asarray(W2, dtype=np.float32)
    pw = np.asarray(pw, dtype=np.float32)
    pb = np.asarray(pb, dtype=np.float32)

    if "nc" not in _BUILD_CACHE:
        _BUILD_CACHE["nc"] = _build_nc(BPC)
    nc = _BUILD_CACHE["nc"]

    in_maps = _host_prep(adj, features, raw_edge_weight, W0, W1, W2, pw, pb)
    res = run_bass_kernel_spmd(
        nc, in_maps, core_ids=list(range(N_CORES)), trace=bool(_trace)
    )
    out = np.concatenate(
        [res.results[c]["out"].reshape(BPC, N_VARS) for c in range(N_CORES)], axis=0
    )
    out = out + pb[None, :].astype(np.float32)
    if _trace:
        return out, res
    return out


# revision 28
# speedup vs baseline: 1.4195x; 1.0356x over previous
"""MASKGCN Trainium2 kernel: 3-layer masked GCN over B=512 graphs of N=200 nodes.

Strategy
--------
Data-parallel over the batch: 64 graphs per NeuronCore, 8 cores, no collectives.

Math fold (exact up to fp reassociation):
    mask = (E + E^T)/2 + I                 (host)
    A    = sigmoid(adj) * mask             (device; adj is 0/1 so
                                            sigmoid(adj) = c*(adj + s), s = 0.5/c,
                                            c = sigmoid(1) - 0.5)
    S0 = F @ W0 ; H1 = A @ S0 ; S1 = H1 @ W1 ; H2 = A @ S1
    out_g = (1/200) * colsum(A)^T @ (H2 @ (W2 @ pw)) + pb
The whole last GCN layer + mean readout + linear head collapse into a
[200]-vector / [256,2]-matrix contraction (colsum(A) is a free-axis reduction
of A^T, fused into the same op that builds A^T).

Precision: all device tensors are fp16 (PE runs fp16 at 1 col/cycle vs
fp32's 4; PSUM accumulation stays fp32). Simulated end-to-end rel-norm
error vs the fp32 reference: ~9e-4.

Layouts: host passes adj^T and F^T per graph. A^T = sigmoid(adj^T) * mask
(mask symmetric). The chain alternates normal/transposed layouts so that NO
on-chip transposes are needed:
    S0  (normal  [node, h])  = matmul(lhsT=F^T slices,  rhs=W0)
    H1t (transp. [h, node])  = matmul(lhsT=S0 slices,   rhs=A^T)
    S1  (normal)             = matmul(lhsT=H1t slices,  rhs=W1)
    H2t (transp.)            = matmul(lhsT=S1 slices,   rhs=A^T)
    S2p (normal [node, 2])   = matmul(lhsT=H2t slices,  rhs=Wp)   Wp=(W2@pw)/200
    og  ([1, 2])             = matmul(lhsT=w,           rhs=S2p)  w=colsum(A)

Engine split: GpSimd builds A^T (SBUF->SBUF, fused colsum accum), Vector
evacuates S0/S1 (PSUM->SBUF), Scalar evacuates H1t/H2t.
"""

import os
import sys
import numpy as np

# concourse is normally pre-imported by the axon sitecustomize; these are
# fallbacks for environments where it is not on the default path.
if "concourse" not in sys.modules:
    try:
        import concourse  # noqa: F401
    except ImportError:
        for _p in ("/opt/trn_rl_repo", "/root/.axon_site/_ro/trn_rl_repo"):
            if os.path.isdir(_p) and _p not in sys.path:
                sys.path.append(_p)

B, N, IN_C, HID, OUT_C, N_VARS = 512, 200, 200, 256, 256, 2
N_CORES = 8
BPC = B // N_CORES  # graphs per core
P0 = 128
P1 = N - P0  # 72

# sigmoid(adj) = C_SIG * (adj + S_SIG) for adj in {0, 1}
C_SIG = float(1.0 / (1.0 + np.exp(-1.0)) - 0.5)  # 0.23105857863000487
S_SIG = float(0.5 / C_SIG)                       # 2.1639534137386535

_BUILD_CACHE = {}


def _build_nc(bpc, reps=1):
    """Build the per-core Bass program (SPMD: identical on all cores).

    reps>1 wraps the whole batch loop in a hardware For_i so the body runs
    `reps` times — benchmarking only (differencing cancels dispatch floor)."""
    import concourse.bacc as bacc
    import concourse.mybir as mybir
    import concourse.tile as tile
    from contextlib import ExitStack

    f32 = mybir.dt.float32
    f16 = mybir.dt.float16
    ADD = mybir.AluOpType.add
    MULT = mybir.AluOpType.mult

    nc = bacc.Bacc(None, target_bir_lowering=False)
    # "adjt" carries host-prebuilt A^T = sigmoid(adj^T)*mask (fp16); "wvt"
    # carries host-prebuilt colsum(A) packed as [128, 2] per graph.
    adjt = nc.declare_dram_parameter("adjt", [bpc, N, N], f16, isOutput=False)
    ft = nc.declare_dram_parameter("ft", [bpc, N, N], f16, isOutput=False)
    wvt = nc.declare_dram_parameter("wvt", [P0, bpc * 2], f16, isOutput=False)
    w0 = nc.declare_dram_parameter("w0", [IN_C, HID], f16, isOutput=False)
    w1 = nc.declare_dram_parameter("w1", [HID, HID], f16, isOutput=False)
    wp = nc.declare_dram_parameter("wp", [OUT_C, N_VARS], f16, isOutput=False)
    out = nc.declare_dram_parameter("out", [1, bpc * N_VARS], f32, isOutput=True)

    with tile.TileContext(nc) as tc, ExitStack() as ctx:
        consts = ctx.enter_context(tc.tile_pool(name="consts", bufs=1))
        inp = ctx.enter_context(tc.tile_pool(name="inp", bufs=12))
        sp = ctx.enter_context(tc.tile_pool(name="sp", bufs=6))
        htp = ctx.enter_context(tc.tile_pool(name="htp", bufs=6))
        smallp = ctx.enter_context(tc.tile_pool(name="smallp", bufs=8))
        pstage = ctx.enter_context(
            tc.tile_pool(name="pstage", bufs=4, space="PSUM")
        )

        # ---- constants (loaded once) ----
        w0a = consts.tile([P0, HID], f16, tag="w0a")
        w0b = consts.tile([P1, HID], f16, tag="w0b")
        w1a = consts.tile([P0, HID], f16, tag="w1a")
        w1b = consts.tile([P0, HID], f16, tag="w1b")
        wpa = consts.tile([P0, N_VARS], f16, tag="wpa")
        wpb = consts.tile([P0, N_VARS], f16, tag="wpb")
        out_acc = consts.tile([1, bpc * N_VARS], f32, tag="out_acc")
        # w0 first: the first graph's S0 needs it; w1/wp aren't read until
        # ~2us later, so they queue behind the first graph DMAs.
        wvall = consts.tile([P0, bpc * 2], f16, tag="wvall")
        nc.sync.dma_start(w0a[:], w0[0:P0, :])
        nc.sync.dma_start(w0b[:], w0[P0:N, :])
        nc.sync.dma_start(wvall[:], wvt[:, :])
        w0_t = (w0a, w0b)
        w1_t = (w1a, w1b)
        wp_t = (wpa, wpb)

        mslc = ((0, P0), (P0, P1))  # node-dim (offset, count) tiles

        def mm_split(out_ap, lhsT_ap, rhs_ap, start, stop):
            """Emit LDWEIGHTS + non-self-loading MATMUL so consecutive
            weight loads pipeline through the PE reorder window instead of
            serializing at the head of every matmul."""
            nc.tensor.ldweights(lhsT_ap)
            inst = nc.tensor.matmul(
                out_ap, lhsT_ap, rhs_ap, start=start, stop=stop
            )
            inst.ldweights = False
            return inst

        def emit_batch():
            state = {}
            late_consts = [False]

            def st_late_consts():
                # Issued after the first pair's input DMAs so they don't
                # delay the first matmul.
                nc.sync.dma_start(w1a[:], w1[0:P0, :])
                nc.sync.dma_start(w1b[:], w1[P0:HID, :])
                nc.sync.dma_start(wpa[:], wp[0:P0, :])
                nc.sync.dma_start(wpb[:], wp[P0:OUT_C, :])

            def st_dma(g):
                t = {}
                t["f0"] = inp.tile([P0, N], f16, tag="f0", name="f0")
                t["f1"] = inp.tile([P1, N], f16, tag="f1", name="f1")
                at0 = inp.tile([P0, N], f16, tag="at0", name="at0")
                at1 = inp.tile([P1, N], f16, tag="at1", name="at1")
                nc.sync.dma_start(t["f0"][:], ft[g, 0:P0, :])
                nc.sync.dma_start(t["f1"][:], ft[g, P0:N, :])
                nc.sync.dma_start(at0[:], adjt[g, 0:P0, :])
                nc.sync.dma_start(at1[:], adjt[g, P0:N, :])
                t["at"] = (at0, at1)
                state[g] = t

            def st_s0(g):
                # S0 = F @ W0 -> psum [node, 2*HID]; single fp16 evacuation
                t = state[g]
                ps0 = pstage.tile([P0, 2 * HID], f32, tag="psA", name="psA")
                f_t = (t["f0"], t["f1"])
                for j, (mo, mc) in enumerate(mslc):
                    for k in range(2):
                        nc.tensor.matmul(
                            ps0[0:mc, j * HID:(j + 1) * HID],
                            f_t[k][:, mo:mo + mc],
                            w0_t[k][:],
                            start=(k == 0), stop=(k == 1),
                        )
                # s01 holds [nodes0:128 x hid | nodes128:200 x hid]; the
                # copy drags along garbage rows 72:128 of the second half.
                s01 = sp.tile([P0, 2 * HID], f16, tag="s01")
                nc.vector.tensor_copy(s01[:], ps0[:])
                t["s01"] = s01

            def st_h1(g):
                # H1^T = matmul(lhsT=S0 slices, rhs=A^T) -> psum [h, 2*N]
                t = state[g]
                ph1 = pstage.tile([P0, 2 * N + 2 * N_VARS + N_VARS], f32,
                                  tag="psB", name="psB")
                s01 = t["s01"]
                for j in range(2):  # h slice
                    for k, kc in ((0, P0), (1, P1)):  # node contraction tile
                        nc.tensor.matmul(
                            ph1[:, j * N:(j + 1) * N],
                            s01[0:kc, k * HID + j * P0:k * HID + (j + 1) * P0],
                            t["at"][k][:],
                            start=(k == 0), stop=(k == 1),
                        )
                h1 = htp.tile([P0, 2 * N], f16, tag="h1")
                nc.scalar.copy(h1[:], ph1[:, 0:2 * N])
                t["h1"] = h1

            def st_s1(g):
                # S1 = H1 @ W1 -> psum [node, 2*HID]
                t = state[g]
                ps1 = pstage.tile([P0, 2 * HID], f32, tag="psA", name="psA")
                h1 = t["h1"]
                for j, (mo, mc) in enumerate(mslc):
                    for k in range(2):  # hid contraction tile
                        nc.tensor.matmul(
                            ps1[0:mc, j * HID:(j + 1) * HID],
                            h1[:, k * N + mo:k * N + mo + mc],
                            w1_t[k][:],
                            start=(k == 0), stop=(k == 1),
                        )
                s11 = sp.tile([P0, 2 * HID], f16, tag="s11")
                nc.vector.tensor_copy(s11[:], ps1[:])
                t["s11"] = s11

            def st_h2(g):
                # H2^T -> psum phx[:, 0:400]; phx also hosts S2p (400:404)
                # and og (404:406) so the whole tail shares one PSUM bank.
                t = state[g]
                phx = pstage.tile([P0, 2 * N + 2 * N_VARS + N_VARS], f32,
                                  tag="psB", name="psB")
                s11 = t["s11"]
                for j in range(2):
                    for k, kc in ((0, P0), (1, P1)):
                        nc.tensor.matmul(
                            phx[:, j * N:(j + 1) * N],
                            s11[0:kc, k * HID + j * P0:k * HID + (j + 1) * P0],
                            t["at"][k][:],
                            start=(k == 0), stop=(k == 1),
                        )
                h2 = htp.tile([P0, 2 * N], f16, tag="h2")
                nc.scalar.copy(h2[:], phx[:, 0:2 * N])
                t["h2"] = h2
                t["phx"] = phx

            def st_tail(g):
                # S2p = H2 @ Wp; og = colsum(A)^T @ S2p
                t = state[g]
                phx = t["phx"]
                h2 = t["h2"]
                c0 = 2 * N
                for j, (mo, mc) in enumerate(mslc):
                    for k in range(2):  # hid contraction tile
                        nc.tensor.matmul(
                            phx[0:mc, c0 + j * N_VARS:c0 + (j + 1) * N_VARS],
                            h2[:, k * N + mo:k * N + mo + mc],
                            wp_t[k][:],
                            start=(k == 0), stop=(k == 1),
                        )
                s2p = smallp.tile([P0, 2 * N_VARS], f16, tag="s2p")
                nc.vector.tensor_copy(s2p[:], phx[:, c0:c0 + 2 * N_VARS])
                t["s2p"] = s2p

            def st_og(g):
                # Deferred one pair: by now the s2p copy has long
                # completed, so these matmuls never stall the PE queue.
                t = state[g]
                phx = t["phx"]
                s2p = t["s2p"]
                c1 = 2 * N + 2 * N_VARS
                nc.tensor.matmul(
                    phx[0:1, c1:c1 + N_VARS], wvall[:, 2 * g:2 * g + 1],
                    s2p[:, 0:N_VARS],
                    start=True, stop=False,
                )
                nc.tensor.matmul(
                    phx[0:1, c1:c1 + N_VARS], wvall[0:P1, 2 * g + 1:2 * g + 2],
                    s2p[0:P1, N_VARS:2 * N_VARS],
                    start=False, stop=True,
                )
                nc.vector.tensor_copy(
                    out_acc[:, g * N_VARS:(g + 1) * N_VARS],
                    phx[0:1, c1:c1 + N_VARS],
                )
                del state[g]

            # Four-graph software pipeline: each stage's PSUM evacuation
            # hides under three partner graphs' matmul streams. S0/S1 share
            # PSUM tag psA, H1/H2/tail share psB (4 bufs each = 8 banks).
            # og matmuls run one quad late so their wait-on-copy never
            # blocks the PE queue.
            GRP = 4
            for q0 in range(0, bpc, GRP):
                G = list(range(q0, q0 + GRP))
                for g in G:
                    st_dma(g)
                if not late_consts[0]:
                    st_late_consts()
                    late_consts[0] = True
                st_s0(G[0])
                st_s0(G[1])
                if q0 >= GRP:
                    for pg in range(q0 - GRP, q0):
                        st_og(pg)
                st_s0(G[2])
                st_s0(G[3])
                for st in (st_h1, st_s1, st_h2, st_tail):
                    for g in G:
                        st(g)
            for pg in range(bpc - GRP, bpc):
                st_og(pg)

        if reps > 1:
            with tc.For_i(0, reps, 1):
                emit_batch()
        else:
            emit_batch()

        nc.sync.dma_start(out[:], out_acc[:])

    nc.compile()
    return nc


def _host_prep(adj, features, raw_edge_weight, W0, W1, W2, pw, pb):
    """Host-side prep: build A^T = sigmoid(adj^T)*mask and colsum(A) here
    (same bytes as shipping adj^T, but saves the on-device mask multiply),
    plus fp16 weight shards."""
    mask = ((raw_edge_weight + raw_edge_weight.T) * 0.5
            + np.eye(N, dtype=np.float32)).astype(np.float32)
    wp = (W2.astype(np.float64) @ pw.astype(np.float64) / float(N)).astype(np.float16)
    w0h = W0.astype(np.float16)
    w1h = W1.astype(np.float16)
    # A^T[g] = (c*adj^T + 0.5) * mask  (mask is symmetric)
    adjt = np.ascontiguousarray(adj.transpose(0, 2, 1))
    at_all = ((np.float32(C_SIG) * adjt + np.float32(0.5)) * mask[None]
              ).astype(np.float16)
    # wv[g, m] = sum_n A^T[g, m, n] = colsum(A)[m]; packed [128, 2] per
    # graph (column 0 = nodes 0:128, column 1 = nodes 128:200 + zero pad),
    # then all graphs side by side: [128, bpc*2].
    wv = at_all.astype(np.float32).sum(axis=2)
    wvt_all = np.zeros((B, P0, 2), dtype=np.float16)
    wvt_all[:, :, 0] = wv[:, 0:P0]
    wvt_all[:, 0:P1, 1] = wv[:, P0:N]
    ft_all = np.ascontiguousarray(
        features.transpose(0, 2, 1).astype(np.float16)
    )
    in_maps = []
    for c in range(N_CORES):
        sl = slice(c * BPC, (c + 1) * BPC)
        in_maps.append({
            "adjt": at_all[sl],
            "ft": ft_all[sl],
            "wvt": np.ascontiguousarray(
                wvt_all[sl].transpose(1, 0, 2).reshape(P0, BPC * 2)
            ),
            "w0": w0h,
            "w1": w1h,
            "wp": wp,
        })
    return in_maps


def _ensure_ntff_hook():
    """Wire the axon NTFF profile hook into antenv.axon_hooks if missing.

    The agent image's antenv package lacks axon_hooks, so bass_utils's
    trace path dies on import. trn_agent_boot has the ctypes hook
    implementation; expose it under the module name bass_utils expects.
    """
    import types

    try:
        from antenv.axon_hooks import get_axon_ntff_profile_hook  # noqa: F401
        return
    except ImportError:
        pass
    try:
        from trn_agent_boot.trn_boot import _ntff_profile_via_ctypes
        hook = _ntff_profile_via_ctypes("/opt/axon/libaxon_pjrt.so")
    except Exception:
        hook = None
    mod = types.ModuleType("antenv.axon_hooks")
    state = {"hook": hook}
    mod.get_axon_ntff_profile_hook = lambda: state["hook"]
    mod.set_axon_ntff_profile_hook = lambda h: state.__setitem__("hook", h)
    sys.modules["antenv.axon_hooks"] = mod
    import antenv

    antenv.axon_hooks = mod


def kernel(adj, features, raw_edge_weight, W0, W1, W2, pw, pb, _trace=False):
    from concourse.bass_utils import run_bass_kernel_spmd

    if _trace:
        _ensure_ntff_hook()

    adj = np.asarray(adj, dtype=np.float32)
    features = np.asarray(features, dtype=np.float32)
    raw_edge_weight = np.asarray(raw_edge_weight, dtype=np.float32)
    W0 = np.asarray(W0, dtype=np.float32)
    W1 = np.asarray(W1, dtype=np.float32)
    W2 = np.asarray(W2, dtype=np.float32)
    pw = np.asarray(pw, dtype=np.float32)
    pb = np.asarray(pb, dtype=np.float32)

    if "nc" not in _BUILD_CACHE:
        _BUILD_CACHE["nc"] = _build_nc(BPC)
    nc = _BUILD_CACHE["nc"]

    in_maps = _host_prep(adj, features, raw_edge_weight, W0, W1, W2, pw, pb)
    res = run_bass_kernel_spmd(
        nc, in_maps, core_ids=list(range(N_CORES)), trace=bool(_trace)
    )
    out = np.concatenate(
        [res.results[c]["out"].reshape(BPC, N_VARS) for c in range(N_CORES)], axis=0
    )
    out = out + pb[None, :].astype(np.float32)
    if _trace:
        return out, res
    return out


# revision 31
# speedup vs baseline: 2.0319x; 1.4314x over previous
"""MASKGCN Trainium2 kernel: 3-layer masked GCN over B=512 graphs of N=200 nodes.

Strategy
--------
Data-parallel over the batch: 64 graphs per NeuronCore, 8 cores, no collectives.

Math fold (exact up to fp reassociation):
    mask = (E + E^T)/2 + I                 (host)
    A    = sigmoid(adj) * mask             (device; adj is 0/1 so
                                            sigmoid(adj) = c*(adj + s), s = 0.5/c,
                                            c = sigmoid(1) - 0.5)
    S0 = F @ W0 ; H1 = A @ S0 ; S1 = H1 @ W1 ; H2 = A @ S1
    out_g = (1/200) * colsum(A)^T @ (H2 @ (W2 @ pw)) + pb
The whole last GCN layer + mean readout + linear head collapse into a
[200]-vector / [256,2]-matrix contraction (colsum(A) is a free-axis reduction
of A^T, fused into the same op that builds A^T).

Precision: all device tensors are fp16 (PE runs fp16 at 1 col/cycle vs
fp32's 4; PSUM accumulation stays fp32). Simulated end-to-end rel-norm
error vs the fp32 reference: ~9e-4.

Layouts: host passes adj^T and F^T per graph. A^T = sigmoid(adj^T) * mask
(mask symmetric). The chain alternates normal/transposed layouts so that NO
on-chip transposes are needed:
    S0  (normal  [node, h])  = matmul(lhsT=F^T slices,  rhs=W0)
    H1t (transp. [h, node])  = matmul(lhsT=S0 slices,   rhs=A^T)
    S1  (normal)             = matmul(lhsT=H1t slices,  rhs=W1)
    H2t (transp.)            = matmul(lhsT=S1 slices,   rhs=A^T)
    S2p (normal [node, 2])   = matmul(lhsT=H2t slices,  rhs=Wp)   Wp=(W2@pw)/200
    og  ([1, 2])             = matmul(lhsT=w,           rhs=S2p)  w=colsum(A)

Engine split: GpSimd builds A^T (SBUF->SBUF, fused colsum accum), Vector
evacuates S0/S1 (PSUM->SBUF), Scalar evacuates H1t/H2t.
"""

import os
import sys
import numpy as np

# concourse is normally pre-imported by the axon sitecustomize; these are
# fallbacks for environments where it is not on the default path.
if "concourse" not in sys.modules:
    try:
        import concourse  # noqa: F401
    except ImportError:
        for _p in ("/opt/trn_rl_repo", "/root/.axon_site/_ro/trn_rl_repo"):
            if os.path.isdir(_p) and _p not in sys.path:
                sys.path.append(_p)

B, N, IN_C, HID, OUT_C, N_VARS = 512, 200, 200, 256, 256, 2
N_CORES = 8
BPC = B // N_CORES  # graphs per core
P0 = 128
P1 = N - P0  # 72

# sigmoid(adj) = C_SIG * (adj + S_SIG) for adj in {0, 1}
C_SIG = float(1.0 / (1.0 + np.exp(-1.0)) - 0.5)  # 0.23105857863000487
S_SIG = float(0.5 / C_SIG)                       # 2.1639534137386535

_BUILD_CACHE = {}


def _build_nc(bpc, reps=1):
    """Build the per-core Bass program (SPMD: identical on all cores).

    reps>1 wraps the whole batch loop in a hardware For_i so the body runs
    `reps` times — benchmarking only (differencing cancels dispatch floor)."""
    import concourse.bacc as bacc
    import concourse.mybir as mybir
    import concourse.tile as tile
    from contextlib import ExitStack

    f32 = mybir.dt.float32
    f16 = mybir.dt.float16
    ADD = mybir.AluOpType.add
    MULT = mybir.AluOpType.mult

    nc = bacc.Bacc(None, target_bir_lowering=False)
    # "fa0"/"fa1" pack F^T and host-prebuilt A^T = sigmoid(adj^T)*mask side
    # by side ([g, row, 0:200] = F^T row, [g, row, 200:400] = A^T row) so
    # each graph needs just two 800B-per-row DMAs. "wvt" carries
    # host-prebuilt colsum(A) packed [128, 2] per graph.
    fa0 = nc.declare_dram_parameter("fa0", [bpc, P0, 2 * N], f16, isOutput=False)
    fa1 = nc.declare_dram_parameter("fa1", [bpc, P1, 2 * N], f16, isOutput=False)
    wvt = nc.declare_dram_parameter("wvt", [P0, bpc * 2], f16, isOutput=False)
    w0 = nc.declare_dram_parameter("w0", [IN_C, HID], f16, isOutput=False)
    w1 = nc.declare_dram_parameter("w1", [HID, HID], f16, isOutput=False)
    wp = nc.declare_dram_parameter("wp", [OUT_C, N_VARS], f16, isOutput=False)
    out = nc.declare_dram_parameter("out", [1, bpc * N_VARS], f32, isOutput=True)

    with tile.TileContext(nc) as tc, ExitStack() as ctx:
        consts = ctx.enter_context(tc.tile_pool(name="consts", bufs=1))
        inp = ctx.enter_context(tc.tile_pool(name="inp", bufs=12))
        sp = ctx.enter_context(tc.tile_pool(name="sp", bufs=6))
        htp = ctx.enter_context(tc.tile_pool(name="htp", bufs=6))
        smallp = ctx.enter_context(tc.tile_pool(name="smallp", bufs=8))
        pstage = ctx.enter_context(
            tc.tile_pool(name="pstage", bufs=4, space="PSUM")
        )

        # ---- constants (loaded once) ----
        w0a = consts.tile([P0, HID], f16, tag="w0a")
        w0b = consts.tile([P1, HID], f16, tag="w0b")
        w1a = consts.tile([P0, HID], f16, tag="w1a")
        w1b = consts.tile([P0, HID], f16, tag="w1b")
        wpa = consts.tile([P0, N_VARS], f16, tag="wpa")
        wpb = consts.tile([P0, N_VARS], f16, tag="wpb")
        out_acc = consts.tile([1, bpc * N_VARS], f32, tag="out_acc")
        # w0 first: the first graph's S0 needs it; w1/wp aren't read until
        # ~2us later, so they queue behind the first graph DMAs.
        wvall = consts.tile([P0, bpc * 2], f16, tag="wvall")
        nc.sync.dma_start(w0a[:], w0[0:P0, :])
        nc.sync.dma_start(w0b[:], w0[P0:N, :])
        nc.sync.dma_start(wvall[:], wvt[:, :])
        w0_t = (w0a, w0b)
        w1_t = (w1a, w1b)
        wp_t = (wpa, wpb)

        mslc = ((0, P0), (P0, P1))  # node-dim (offset, count) tiles

        def mm_split(out_ap, lhsT_ap, rhs_ap, start, stop):
            """Emit LDWEIGHTS + non-self-loading MATMUL so consecutive
            weight loads pipeline through the PE reorder window instead of
            serializing at the head of every matmul."""
            nc.tensor.ldweights(lhsT_ap)
            inst = nc.tensor.matmul(
                out_ap, lhsT_ap, rhs_ap, start=start, stop=stop
            )
            inst.ldweights = False
            return inst

        def emit_batch():
            state = {}

            def st_late_consts():
                # Issued after the first pair's input DMAs so they don't
                # delay the first matmul.
                nc.sync.dma_start(w1a[:], w1[0:P0, :])
                nc.sync.dma_start(w1b[:], w1[P0:HID, :])
                nc.sync.dma_start(wpa[:], wp[0:P0, :])
                nc.sync.dma_start(wpb[:], wp[P0:OUT_C, :])

            def st_dma(g):
                t = {}
                fa0t = inp.tile([P0, 2 * N], f16, tag="fa0", name="fa0")
                fa1t = inp.tile([P1, 2 * N], f16, tag="fa1", name="fa1")
                nc.sync.dma_start(fa0t[:], fa0[g, :, :])
                nc.sync.dma_start(fa1t[:], fa1[g, :, :])
                t["fa"] = (fa0t, fa1t)
                state[g] = t

            def st_s0(g):
                # S0 = F @ W0 -> psum [node, 2*HID]; single fp16 evacuation
                t = state[g]
                ps0 = pstage.tile([P0, 2 * HID], f32, tag="psA", name="psA")
                fa = t["fa"]
                for j, (mo, mc) in enumerate(mslc):
                    for k in range(2):
                        nc.tensor.matmul(
                            ps0[0:mc, j * HID:(j + 1) * HID],
                            fa[k][:, mo:mo + mc],
                            w0_t[k][:],
                            start=(k == 0), stop=(k == 1),
                        )
                # s01 holds [nodes0:128 x hid | nodes128:200 x hid]; the
                # copy drags along garbage rows 72:128 of the second half.
                s01 = sp.tile([P0, 2 * HID], f16, tag="s01")
                nc.vector.tensor_copy(s01[:], ps0[:])
                t["s01"] = s01

            def st_h1(g):
                # H1^T = matmul(lhsT=S0 slices, rhs=A^T) -> psum [h, 2*N]
                t = state[g]
                ph1 = pstage.tile([P0, 2 * N + 2 * N_VARS + N_VARS], f32,
                                  tag="psB", name="psB")
                s01 = t["s01"]
                for j in range(2):  # h slice
                    for k, kc in ((0, P0), (1, P1)):  # node contraction tile
                        nc.tensor.matmul(
                            ph1[:, j * N:(j + 1) * N],
                            s01[0:kc, k * HID + j * P0:k * HID + (j + 1) * P0],
                            t["fa"][k][:, N:2 * N],
                            start=(k == 0), stop=(k == 1),
                        )
                h1 = htp.tile([P0, 2 * N], f16, tag="h1")
                nc.scalar.copy(h1[:], ph1[:, 0:2 * N])
                t["h1"] = h1

            def st_s1(g):
                # S1 = H1 @ W1 -> psum [node, 2*HID]
                t = state[g]
                ps1 = pstage.tile([P0, 2 * HID], f32, tag="psA", name="psA")
                h1 = t["h1"]
                for j, (mo, mc) in enumerate(mslc):
                    for k in range(2):  # hid contraction tile
                        nc.tensor.matmul(
                            ps1[0:mc, j * HID:(j + 1) * HID],
                            h1[:, k * N + mo:k * N + mo + mc],
                            w1_t[k][:],
                            start=(k == 0), stop=(k == 1),
                        )
                s11 = sp.tile([P0, 2 * HID], f16, tag="s11")
                nc.vector.tensor_copy(s11[:], ps1[:])
                t["s11"] = s11

            def st_h2(g):
                # H2^T -> psum phx[:, 0:400]; phx also hosts S2p (400:404)
                # and og (404:406) so the whole tail shares one PSUM bank.
                t = state[g]
                phx = pstage.tile([P0, 2 * N + 2 * N_VARS + N_VARS], f32,
                                  tag="psB", name="psB")
                s11 = t["s11"]
                for j in range(2):
                    for k, kc in ((0, P0), (1, P1)):
                        nc.tensor.matmul(
                            phx[:, j * N:(j + 1) * N],
                            s11[0:kc, k * HID + j * P0:k * HID + (j + 1) * P0],
                            t["fa"][k][:, N:2 * N],
                            start=(k == 0), stop=(k == 1),
                        )
                h2 = htp.tile([P0, 2 * N], f16, tag="h2")
                nc.scalar.copy(h2[:], phx[:, 0:2 * N])
                t["h2"] = h2
                t["phx"] = phx

            def st_tail(g):
                # S2p = H2 @ Wp; og = colsum(A)^T @ S2p
                t = state[g]
                phx = t["phx"]
                h2 = t["h2"]
                c0 = 2 * N
                for j, (mo, mc) in enumerate(mslc):
                    for k in range(2):  # hid contraction tile
                        nc.tensor.matmul(
                            phx[0:mc, c0 + j * N_VARS:c0 + (j + 1) * N_VARS],
                            h2[:, k * N + mo:k * N + mo + mc],
                            wp_t[k][:],
                            start=(k == 0), stop=(k == 1),
                        )
                s2p = smallp.tile([P0, 2 * N_VARS], f16, tag="s2p")
                nc.vector.tensor_copy(s2p[:], phx[:, c0:c0 + 2 * N_VARS])
                t["s2p"] = s2p

            def st_og(g):
                # Deferred one pair: by now the s2p copy has long
                # completed, so these matmuls never stall the PE queue.
                t = state[g]
                phx = t["phx"]
                s2p = t["s2p"]
                c1 = 2 * N + 2 * N_VARS
                nc.tensor.matmul(
                    phx[0:1, c1:c1 + N_VARS], wvall[:, 2 * g:2 * g + 1],
                    s2p[:, 0:N_VARS],
                    start=True, stop=False,
                )
                nc.tensor.matmul(
                    phx[0:1, c1:c1 + N_VARS], wvall[0:P1, 2 * g + 1:2 * g + 2],
                    s2p[0:P1, N_VARS:2 * N_VARS],
                    start=False, stop=True,
                )
                nc.vector.tensor_copy(
                    out_acc[:, g * N_VARS:(g + 1) * N_VARS],
                    phx[0:1, c1:c1 + N_VARS],
                )
                del state[g]

            # Four-graph software pipeline: each stage's PSUM evacuation
            # hides under three partner graphs' matmul streams. S0/S1 share
            # PSUM tag psA, H1/H2/tail share psB (4 bufs each = 8 banks).
            # og matmuls run one quad late so their wait-on-copy never
            # blocks the PE queue.
            GRP = 4
            for q0 in range(0, bpc, GRP):
                G = list(range(q0, q0 + GRP))
                if q0 == 0:
                    for g in range(0, min(2 * GRP, bpc)):
                        st_dma(g)
                    st_late_consts()
                for g in range(q0 + 2 * GRP, min(q0 + 3 * GRP, bpc)):
                    st_dma(g)
                st_s0(G[0])
                st_s0(G[1])
                if q0 >= GRP:
                    for pg in range(q0 - GRP, q0):
                        st_og(pg)
                st_s0(G[2])
                st_s0(G[3])
                for st in (st_h1, st_s1, st_h2, st_tail):
                    for g in G:
                        st(g)
            for pg in range(bpc - GRP, bpc):
                st_og(pg)

        if reps > 1:
            with tc.For_i(0, reps, 1):
                emit_batch()
        else:
            emit_batch()

        nc.sync.dma_start(out[:], out_acc[:])

    nc.compile()
    return nc


def _host_prep(adj, features, raw_edge_weight, W0, W1, W2, pw, pb):
    """Host-side prep: build A^T = sigmoid(adj^T)*mask and colsum(A) here
    (same bytes as shipping adj^T, but saves the on-device mask multiply),
    plus fp16 weight shards."""
    mask = ((raw_edge_weight + raw_edge_weight.T) * 0.5
            + np.eye(N, dtype=np.float32)).astype(np.float32)
    wp = (W2.astype(np.float64) @ pw.astype(np.float64) / float(N)).astype(np.float16)
    w0h = W0.astype(np.float16)
    w1h = W1.astype(np.float16)
    # A^T[g] = (c*adj^T + 0.5) * mask  (mask is symmetric)
    adjt = np.ascontiguousarray(adj.transpose(0, 2, 1))
    at_all = ((np.float32(C_SIG) * adjt + np.float32(0.5)) * mask[None]
              ).astype(np.float16)
    ft16 = features.transpose(0, 2, 1).astype(np.float16)
    fa_all = np.concatenate([ft16, at_all], axis=2)  # [B, N, 2N]
    # wv[g, m] = sum_n A^T[g, m, n] = colsum(A)[m]; packed [128, 2] per
    # graph (column 0 = nodes 0:128, column 1 = nodes 128:200 + zero pad),
    # then all graphs side by side: [128, bpc*2].
    wv = at_all.astype(np.float32).sum(axis=2)
    wvt_all = np.zeros((B, P0, 2), dtype=np.float16)
    wvt_all[:, :, 0] = wv[:, 0:P0]
    wvt_all[:, 0:P1, 1] = wv[:, P0:N]
    in_maps = []
    for c in range(N_CORES):
        sl = slice(c * BPC, (c + 1) * BPC)
        in_maps.append({
            "fa0": np.ascontiguousarray(fa_all[sl, 0:P0, :]),
            "fa1": np.ascontiguousarray(fa_all[sl, P0:N, :]),
            "wvt": np.ascontiguousarray(
                wvt_all[sl].transpose(1, 0, 2).reshape(P0, BPC * 2)
            ),
            "w0": w0h,
            "w1": w1h,
            "wp": wp,
        })
    return in_maps


def _ensure_ntff_hook():
    """Wire the axon NTFF profile hook into antenv.axon_hooks if missing.

    The agent image's antenv package lacks axon_hooks, so bass_utils's
    trace path dies on import. trn_agent_boot has the ctypes hook
    implementation; expose it under the module name bass_utils expects.
    """
    import types

    try:
        from antenv.axon_hooks import get_axon_ntff_profile_hook  # noqa: F401
        return
    except ImportError:
        pass
    try:
        from trn_agent_boot.trn_boot import _ntff_profile_via_ctypes
        hook = _ntff_profile_via_ctypes("/opt/axon/libaxon_pjrt.so")
    except Exception:
        hook = None
    mod = types.ModuleType("antenv.axon_hooks")
    state = {"hook": hook}
    mod.get_axon_ntff_profile_hook = lambda: state["hook"]
    mod.set_axon_ntff_profile_hook = lambda h: state.__setitem__("hook", h)
    sys.modules["antenv.axon_hooks"] = mod
    import antenv

    antenv.axon_hooks = mod


def kernel(adj, features, raw_edge_weight, W0, W1, W2, pw, pb, _trace=False):
    from concourse.bass_utils import run_bass_kernel_spmd

    if _trace:
        _ensure_ntff_hook()

    adj = np.asarray(adj, dtype=np.float32)
    features = np.asarray(features, dtype=np.float32)
    raw_edge_weight = np.asarray(raw_edge_weight, dtype=np.float32)
    W0 = np.asarray(W0, dtype=np.float32)
    W1 = np.asarray(W1, dtype=np.float32)
    W2 = np.asarray(W2, dtype=np.float32)
    pw = np.asarray(pw, dtype=np.float32)
    pb = np.asarray(pb, dtype=np.float32)

    if "nc" not in _BUILD_CACHE:
        _BUILD_CACHE["nc"] = _build_nc(BPC)
    nc = _BUILD_CACHE["nc"]

    in_maps = _host_prep(adj, features, raw_edge_weight, W0, W1, W2, pw, pb)
    res = run_bass_kernel_spmd(
        nc, in_maps, core_ids=list(range(N_CORES)), trace=bool(_trace)
    )
    out = np.concatenate(
        [res.results[c]["out"].reshape(BPC, N_VARS) for c in range(N_CORES)], axis=0
    )
    out = out + pb[None, :].astype(np.float32)
    if _trace:
        return out, res
    return out


# revision 33
# speedup vs baseline: 2.1082x; 1.0376x over previous
"""MASKGCN Trainium2 kernel: 3-layer masked GCN over B=512 graphs of N=200 nodes.

Strategy
--------
Data-parallel over the batch: 64 graphs per NeuronCore, 8 cores, no collectives.

Math fold (exact up to fp reassociation):
    mask = (E + E^T)/2 + I                 (host)
    A    = sigmoid(adj) * mask             (device; adj is 0/1 so
                                            sigmoid(adj) = c*(adj + s), s = 0.5/c,
                                            c = sigmoid(1) - 0.5)
    S0 = F @ W0 ; H1 = A @ S0 ; S1 = H1 @ W1 ; H2 = A @ S1
    out_g = (1/200) * colsum(A)^T @ (H2 @ (W2 @ pw)) + pb
The whole last GCN layer + mean readout + linear head collapse into a
[200]-vector / [256,2]-matrix contraction (colsum(A) is a free-axis reduction
of A^T, fused into the same op that builds A^T).

Precision: all device tensors are fp16 (PE runs fp16 at 1 col/cycle vs
fp32's 4; PSUM accumulation stays fp32). Simulated end-to-end rel-norm
error vs the fp32 reference: ~9e-4.

Layouts: host passes adj^T and F^T per graph. A^T = sigmoid(adj^T) * mask
(mask symmetric). The chain alternates normal/transposed layouts so that NO
on-chip transposes are needed:
    S0  (normal  [node, h])  = matmul(lhsT=F^T slices,  rhs=W0)
    H1t (transp. [h, node])  = matmul(lhsT=S0 slices,   rhs=A^T)
    S1  (normal)             = matmul(lhsT=H1t slices,  rhs=W1)
    H2t (transp.)            = matmul(lhsT=S1 slices,   rhs=A^T)
    S2p (normal [node, 2])   = matmul(lhsT=H2t slices,  rhs=Wp)   Wp=(W2@pw)/200
    og  ([1, 2])             = matmul(lhsT=w,           rhs=S2p)  w=colsum(A)

Engine split: GpSimd builds A^T (SBUF->SBUF, fused colsum accum), Vector
evacuates S0/S1 (PSUM->SBUF), Scalar evacuates H1t/H2t.
"""

import os
import sys
import numpy as np

# concourse is normally pre-imported by the axon sitecustomize; these are
# fallbacks for environments where it is not on the default path.
if "concourse" not in sys.modules:
    try:
        import concourse  # noqa: F401
    except ImportError:
        for _p in ("/opt/trn_rl_repo", "/root/.axon_site/_ro/trn_rl_repo"):
            if os.path.isdir(_p) and _p not in sys.path:
                sys.path.append(_p)

B, N, IN_C, HID, OUT_C, N_VARS = 512, 200, 200, 256, 256, 2
N_CORES = 8
BPC = B // N_CORES  # graphs per core
P0 = 128
P1 = N - P0  # 72

# sigmoid(adj) = C_SIG * (adj + S_SIG) for adj in {0, 1}
C_SIG = float(1.0 / (1.0 + np.exp(-1.0)) - 0.5)  # 0.23105857863000487
S_SIG = float(0.5 / C_SIG)                       # 2.1639534137386535

_BUILD_CACHE = {}


def _build_nc(bpc, reps=1):
    """Build the per-core Bass program (SPMD: identical on all cores).

    reps>1 wraps the whole batch loop in a hardware For_i so the body runs
    `reps` times — benchmarking only (differencing cancels dispatch floor)."""
    import concourse.bacc as bacc
    import concourse.mybir as mybir
    import concourse.tile as tile
    from contextlib import ExitStack

    f32 = mybir.dt.float32
    f16 = mybir.dt.float16
    ADD = mybir.AluOpType.add
    MULT = mybir.AluOpType.mult

    nc = bacc.Bacc(None, target_bir_lowering=False)
    # "fa0"/"fa1" pack F^T and host-prebuilt A^T = sigmoid(adj^T)*mask side
    # by side ([g, row, 0:200] = F^T row, [g, row, 200:400] = A^T row) so
    # each graph needs just two 800B-per-row DMAs. "wvt" carries
    # host-prebuilt colsum(A) packed [128, 2] per graph.
    fa0 = nc.declare_dram_parameter("fa0", [bpc, P0, 2 * N], f16, isOutput=False)
    fa1 = nc.declare_dram_parameter("fa1", [bpc, P1, 2 * N], f16, isOutput=False)
    wvt = nc.declare_dram_parameter("wvt", [P0, bpc * 2], f16, isOutput=False)
    w0 = nc.declare_dram_parameter("w0", [IN_C, HID], f16, isOutput=False)
    w1 = nc.declare_dram_parameter("w1", [HID, HID], f16, isOutput=False)
    wp = nc.declare_dram_parameter("wp", [OUT_C, N_VARS], f16, isOutput=False)
    out = nc.declare_dram_parameter("out", [1, bpc * N_VARS], f32, isOutput=True)

    with tile.TileContext(nc) as tc, ExitStack() as ctx:
        consts = ctx.enter_context(tc.tile_pool(name="consts", bufs=1))
        inp = ctx.enter_context(tc.tile_pool(name="inp", bufs=12))
        sp = ctx.enter_context(tc.tile_pool(name="sp", bufs=6))
        htp = ctx.enter_context(tc.tile_pool(name="htp", bufs=6))
        smallp = ctx.enter_context(tc.tile_pool(name="smallp", bufs=8))
        pstage = ctx.enter_context(
            tc.tile_pool(name="pstage", bufs=4, space="PSUM")
        )

        # ---- constants (loaded once) ----
        w0a = consts.tile([P0, HID], f16, tag="w0a")
        w0b = consts.tile([P1, HID], f16, tag="w0b")
        w1a = consts.tile([P0, HID], f16, tag="w1a")
        w1b = consts.tile([P0, HID], f16, tag="w1b")
        wpa = consts.tile([P0, N_VARS], f16, tag="wpa")
        wpb = consts.tile([P0, N_VARS], f16, tag="wpb")
        out_acc = consts.tile([1, bpc * N_VARS], f32, tag="out_acc")
        # w0 first: the first graph's S0 needs it; w1/wp aren't read until
        # ~2us later, so they queue behind the first graph DMAs.
        wvall = consts.tile([P0, bpc * 2], f16, tag="wvall")
        nc.sync.dma_start(w0a[:], w0[0:P0, :])
        nc.sync.dma_start(w0b[:], w0[P0:N, :])
        w0_t = (w0a, w0b)
        w1_t = (w1a, w1b)
        wp_t = (wpa, wpb)

        mslc = ((0, P0), (P0, P1))  # node-dim (offset, count) tiles

        def mm_split(out_ap, lhsT_ap, rhs_ap, start, stop):
            """Emit LDWEIGHTS + non-self-loading MATMUL so consecutive
            weight loads pipeline through the PE reorder window instead of
            serializing at the head of every matmul."""
            nc.tensor.ldweights(lhsT_ap)
            inst = nc.tensor.matmul(
                out_ap, lhsT_ap, rhs_ap, start=start, stop=stop
            )
            inst.ldweights = False
            return inst

        def emit_batch():
            state = {}

            def st_late_consts():
                # Issued after the first pair's input DMAs so they don't
                # delay the first matmul.
                nc.sync.dma_start(w1a[:], w1[0:P0, :])
                nc.sync.dma_start(w1b[:], w1[P0:HID, :])
                nc.sync.dma_start(wpa[:], wp[0:P0, :])
                nc.sync.dma_start(wpb[:], wp[P0:OUT_C, :])
                nc.sync.dma_start(wvall[:], wvt[:, :])

            def st_dma(g):
                t = {}
                fa0t = inp.tile([P0, 2 * N], f16, tag="fa0", name="fa0")
                fa1t = inp.tile([P1, 2 * N], f16, tag="fa1", name="fa1")
                nc.sync.dma_start(fa0t[:], fa0[g, :, :])
                nc.sync.dma_start(fa1t[:], fa1[g, :, :])
                t["fa"] = (fa0t, fa1t)
                state[g] = t

            def st_s0(g):
                # S0 = F @ W0 -> psum [node, 2*HID]; single fp16 evacuation
                t = state[g]
                ps0 = pstage.tile([P0, 2 * HID], f32, tag="psA", name="psA")
                fa = t["fa"]
                for j, (mo, mc) in enumerate(mslc):
                    for k in range(2):
                        nc.tensor.matmul(
                            ps0[0:mc, j * HID:(j + 1) * HID],
                            fa[k][:, mo:mo + mc],
                            w0_t[k][:],
                            start=(k == 0), stop=(k == 1),
                        )
                # s01 holds [nodes0:128 x hid | nodes128:200 x hid]; the
                # copy drags along garbage rows 72:128 of the second half.
                s01 = sp.tile([P0, 2 * HID], f16, tag="s01")
                nc.vector.tensor_copy(s01[:], ps0[:])
                t["s01"] = s01

            def st_h1(g):
                # H1^T = matmul(lhsT=S0 slices, rhs=A^T) -> psum [h, 2*N]
                t = state[g]
                ph1 = pstage.tile([P0, 2 * N + 2 * N_VARS + N_VARS], f32,
                                  tag="psB", name="psB")
                s01 = t["s01"]
                for j in range(2):  # h slice
                    for k, kc in ((0, P0), (1, P1)):  # node contraction tile
                        nc.tensor.matmul(
                            ph1[:, j * N:(j + 1) * N],
                            s01[0:kc, k * HID + j * P0:k * HID + (j + 1) * P0],
                            t["fa"][k][:, N:2 * N],
                            start=(k == 0), stop=(k == 1),
                        )
                h1 = htp.tile([P0, 2 * N], f16, tag="h1")
                nc.scalar.copy(h1[:], ph1[:, 0:2 * N])
                t["h1"] = h1

            def st_s1(g):
                # S1 = H1 @ W1 -> psum [node, 2*HID]
                t = state[g]
                ps1 = pstage.tile([P0, 2 * HID], f32, tag="psA", name="psA")
                h1 = t["h1"]
                for j, (mo, mc) in enumerate(mslc):
                    for k in range(2):  # hid contraction tile
                        nc.tensor.matmul(
                            ps1[0:mc, j * HID:(j + 1) * HID],
                            h1[:, k * N + mo:k * N + mo + mc],
                            w1_t[k][:],
                            start=(k == 0), stop=(k == 1),
                        )
                s11 = sp.tile([P0, 2 * HID], f16, tag="s11")
                nc.vector.tensor_copy(s11[:], ps1[:])
                t["s11"] = s11

            def st_h2(g):
                # H2^T -> psum phx[:, 0:400]; phx also hosts S2p (400:404)
                # and og (404:406) so the whole tail shares one PSUM bank.
                t = state[g]
                phx = pstage.tile([P0, 2 * N + 2 * N_VARS + N_VARS], f32,
                                  tag="psB", name="psB")
                s11 = t["s11"]
                for j in range(2):
                    for k, kc in ((0, P0), (1, P1)):
                        nc.tensor.matmul(
                            phx[:, j * N:(j + 1) * N],
                            s11[0:kc, k * HID + j * P0:k * HID + (j + 1) * P0],
                            t["fa"][k][:, N:2 * N],
                            start=(k == 0), stop=(k == 1),
                        )
                h2 = htp.tile([P0, 2 * N], f16, tag="h2")
                nc.scalar.copy(h2[:], phx[:, 0:2 * N])
                t["h2"] = h2
                t["phx"] = phx

            def st_tail(g):
                # S2p = H2 @ Wp; og = colsum(A)^T @ S2p
                t = state[g]
                phx = t["phx"]
                h2 = t["h2"]
                c0 = 2 * N
                for j, (mo, mc) in enumerate(mslc):
                    for k in range(2):  # hid contraction tile
                        nc.tensor.matmul(
                            phx[0:mc, c0 + j * N_VARS:c0 + (j + 1) * N_VARS],
                            h2[:, k * N + mo:k * N + mo + mc],
                            wp_t[k][:],
                            start=(k == 0), stop=(k == 1),
                        )
                s2p = smallp.tile([P0, 2 * N_VARS], f16, tag="s2p")
                nc.vector.tensor_copy(s2p[:], phx[:, c0:c0 + 2 * N_VARS])
                t["s2p"] = s2p

            def st_og(g):
                # Deferred one pair: by now the s2p copy has long
                # completed, so these matmuls never stall the PE queue.
                t = state[g]
                phx = t["phx"]
                s2p = t["s2p"]
                c1 = 2 * N + 2 * N_VARS
                nc.tensor.matmul(
                    phx[0:1, c1:c1 + N_VARS], wvall[:, 2 * g:2 * g + 1],
                    s2p[:, 0:N_VARS],
                    start=True, stop=False,
                )
                nc.tensor.matmul(
                    phx[0:1, c1:c1 + N_VARS], wvall[0:P1, 2 * g + 1:2 * g + 2],
                    s2p[0:P1, N_VARS:2 * N_VARS],
                    start=False, stop=True,
                )
                nc.vector.tensor_copy(
                    out_acc[:, g * N_VARS:(g + 1) * N_VARS],
                    phx[0:1, c1:c1 + N_VARS],
                )
                del state[g]

            # Four-graph software pipeline: each stage's PSUM evacuation
            # hides under three partner graphs' matmul streams. S0/S1 share
            # PSUM tag psA, H1/H2/tail share psB (4 bufs each = 8 banks).
            # og matmuls run one quad late so their wait-on-copy never
            # blocks the PE queue.
            GRP = 4
            for q0 in range(0, bpc, GRP):
                G = list(range(q0, q0 + GRP))
                if q0 == 0:
                    # Minimal critical prefix: only quad 0's inputs go
                    # ahead of the first matmuls (each dma_start costs
                    # ~360ns of sync-queue dispatch).
                    for g in G:
                        st_dma(g)
                st_s0(G[0])
                st_s0(G[1])
                if q0 == 0:
                    st_late_consts()
                    for g in range(GRP, 2 * GRP):
                        st_dma(g)
                else:
                    for pg in range(q0 - GRP, q0):
                        st_og(pg)
                st_s0(G[2])
                st_s0(G[3])
                for g in range(q0 + 2 * GRP, min(q0 + 3 * GRP, bpc)):
                    st_dma(g)
                for st in (st_h1, st_s1, st_h2, st_tail):
                    for g in G:
                        st(g)
            for pg in range(bpc - GRP, bpc):
                st_og(pg)

        if reps > 1:
            with tc.For_i(0, reps, 1):
                emit_batch()
        else:
            emit_batch()

        nc.sync.dma_start(out[:], out_acc[:])

    nc.compile()
    return nc


def _host_prep(adj, features, raw_edge_weight, W0, W1, W2, pw, pb):
    """Host-side prep: build A^T = sigmoid(adj^T)*mask and colsum(A) here
    (same bytes as shipping adj^T, but saves the on-device mask multiply),
    plus fp16 weight shards."""
    mask = ((raw_edge_weight + raw_edge_weight.T) * 0.5
            + np.eye(N, dtype=np.float32)).astype(np.float32)
    wp = (W2.astype(np.float64) @ pw.astype(np.float64) / float(N)).astype(np.float16)
    w0h = W0.astype(np.float16)
    w1h = W1.astype(np.float16)
    # A^T[g] = (c*adj^T + 0.5) * mask  (mask is symmetric)
    adjt = np.ascontiguousarray(adj.transpose(0, 2, 1))
    at_all = ((np.float32(C_SIG) * adjt + np.float32(0.5)) * mask[None]
              ).astype(np.float16)
    ft16 = features.transpose(0, 2, 1).astype(np.float16)
    fa_all = np.concatenate([ft16, at_all], axis=2)  # [B, N, 2N]
    # wv[g, m] = sum_n A^T[g, m, n] = colsum(A)[m]; packed [128, 2] per
    # graph (column 0 = nodes 0:128, column 1 = nodes 128:200 + zero pad),
    # then all graphs side by side: [128, bpc*2].
    wv = at_all.astype(np.float32).sum(axis=2)
    wvt_all = np.zeros((B, P0, 2), dtype=np.float16)
    wvt_all[:, :, 0] = wv[:, 0:P0]
    wvt_all[:, 0:P1, 1] = wv[:, P0:N]
    in_maps = []
    for c in range(N_CORES):
        sl = slice(c * BPC, (c + 1) * BPC)
        in_maps.append({
            "fa0": np.ascontiguousarray(fa_all[sl, 0:P0, :]),
            "fa1": np.ascontiguousarray(fa_all[sl, P0:N, :]),
            "wvt": np.ascontiguousarray(
                wvt_all[sl].transpose(1, 0, 2).reshape(P0, BPC * 2)
            ),
            "w0": w0h,
            "w1": w1h,
            "wp": wp,
        })
    return in_maps


def _ensure_ntff_hook():
    """Wire the axon NTFF profile hook into antenv.axon_hooks if missing.

    The agent image's antenv package lacks axon_hooks, so bass_utils's
    trace path dies on import. trn_agent_boot has the ctypes hook
    implementation; expose it under the module name bass_utils expects.
    """
    import types

    try:
        from antenv.axon_hooks import get_axon_ntff_profile_hook  # noqa: F401
        return
    except ImportError:
        pass
    try:
        from trn_agent_boot.trn_boot import _ntff_profile_via_ctypes
        hook = _ntff_profile_via_ctypes("/opt/axon/libaxon_pjrt.so")
    except Exception:
        hook = None
    mod = types.ModuleType("antenv.axon_hooks")
    state = {"hook": hook}
    mod.get_axon_ntff_profile_hook = lambda: state["hook"]
    mod.set_axon_ntff_profile_hook = lambda h: state.__setitem__("hook", h)
    sys.modules["antenv.axon_hooks"] = mod
    import antenv

    antenv.axon_hooks = mod


def kernel(adj, features, raw_edge_weight, W0, W1, W2, pw, pb, _trace=False):
    from concourse.bass_utils import run_bass_kernel_spmd

    if _trace:
        _ensure_ntff_hook()

    adj = np.asarray(adj, dtype=np.float32)
    features = np.asarray(features, dtype=np.float32)
    raw_edge_weight = np.asarray(raw_edge_weight, dtype=np.float32)
    W0 = np.asarray(W0, dtype=np.float32)
    W1 = np.asarray(W1, dtype=np.float32)
    W2 = np.asarray(W2, dtype=np.float32)
    pw = np.asarray(pw, dtype=np.float32)
    pb = np.asarray(pb, dtype=np.float32)

    if "nc" not in _BUILD_CACHE:
        _BUILD_CACHE["nc"] = _build_nc(BPC)
    nc = _BUILD_CACHE["nc"]

    in_maps = _host_prep(adj, features, raw_edge_weight, W0, W1, W2, pw, pb)
    res = run_bass_kernel_spmd(
        nc, in_maps, core_ids=list(range(N_CORES)), trace=bool(_trace)
    )
    out = np.concatenate(
        [res.results[c]["out"].reshape(BPC, N_VARS) for c in range(N_CORES)], axis=0
    )
    out = out + pb[None, :].astype(np.float32)
    if _trace:
        return out, res
    return out
